# revision 30
# baseline (speedup 1.0000x reference)
"""Causal self-attention block (qkv proj + 16-head causal attention + out_proj
+ c_proj) on 8 trn2 NeuronCores, data-parallel over the batch (B=8: one batch
element per core).

Layout strategy (per core, batch element b):
  - Activations are kept feature-major [feature, token] on chip so every
    linear layer is a plain   out = W_T.T @ act   matmul chain with the
    (host-pre-transposed) weight as the stationary operand. No on-device
    transposes at all.
  - All projection matmuls run in bf16 (1 PE cycle/row; verified 3.8e-3 rel
    error on the full pipeline vs the 2e-2 budget); PSUM accumulates fp32.
  - Attention computes transposed scores  sT[tk, tq] = k_h.T q_h  per head
    pair (row-tiled K=64 matmuls), exp with no max-subtraction (scores here
    are bounded by a few units), causal mask as a bf16 multiply on the
    diagonal blocks on the otherwise-idle gpsimd engine, and the AV product
    consumes sT directly with token-major V tiles as the stationary operand.
    A fused ones-row in the V operand (M=65) yields the softmax denominator
    for free.
  - Denominator rows are DMA-scattered into per-half [8, T] tiles; 1/den via
    the fast approx-reciprocal DVE op, partition-broadcast by K=8 indicator
    matmuls into PSUM, and normalized into y by DVE multiplies. Each
    half-chunk normalizes as soon as its 4 attention pairs finish, so only
    the (chunk1, heads 8-15) normalization sits on the critical tail.
  - V bias and out_proj bias are folded into the c_proj bias host-side
    (exact algebra), so only the qk bias is applied on chip.
  - Schedule: S1 computes qk pairs 0-3 + V heads 0-7; S2 runs attention for
    pairs 0-3 (both chunks) and pairs 4-7 (chunk 0) over the remaining
    qkv projection work; S3 runs attention pairs 4-7 (chunk 1) over
    out_proj + c_proj of chunk 0; S4 drains out_proj + c_proj of chunk 1.
"""

import sys

if "/opt/trn_rl_repo" not in sys.path:
    sys.path.insert(0, "/opt/trn_rl_repo")

import ml_dtypes
import numpy as np

import concourse.bass as bass  # noqa: F401  (bass types used via tile/bacc)
import concourse.tile as tile
from concourse import bacc, mybir
from concourse.bass_utils import run_bass_kernel_spmd
from concourse.dve_ops import (
    RECIP_APPROX_FAST_CONSTS as RC,
    RECIPROCAL_APPROX_FAST,
)

B, T, E, H = 8, 1024, 1024, 16
DH = E // H          # 64
JQK = 2 * E          # q+k fused feature dim (2048)
F32 = mybir.dt.float32
F32R = mybir.dt.float32r
BF16 = mybir.dt.bfloat16
Act = mybir.ActivationFunctionType

TRACE = False        # test harness flips this for profiled runs
_CACHE = {}


def _emit(nc, tc, aps):
    (xT, wqkT, wvT, bqk, woutT, wcT, bc, mask01, ind, onesbf, outT) = aps
    ET = E // 128     # 8  e-tiles (contraction)
    TT = T // 128     # 8  token tiles
    NT = T // 512     # 2  512-wide token column chunks

    consts = tc.alloc_tile_pool(name="consts", bufs=1)
    mask01b = consts.tile([128, 128], BF16, tag="mask01b")
    bqkb = consts.tile([128, JQK // 128], F32, tag="bqkb")
    bcb = consts.tile([128, E // 128], F32, tag="bcb")
    indb = consts.tile([8, (H // 4) * 128], F32R, tag="indb")

    psum = tc.alloc_tile_pool(name="psum", bufs=1, space="PSUM")
    p_den = tc.alloc_tile_pool(name="p_den", bufs=1)
    p_y = tc.alloc_tile_pool(name="p_y", bufs=1)
    p_qk = tc.alloc_tile_pool(name="p_qk", bufs=1)
    p_v = tc.alloc_tile_pool(name="p_v", bufs=1)
    p_x = tc.alloc_tile_pool(name="p_x", bufs=1)
    p_wqk = tc.alloc_tile_pool(name="p_wqk", bufs=16)
    den = [p_den.tile([8, T], F32, tag=f"den{i}", name=f"den{i}")
           for i in range(2)]
    rec = [p_den.tile([8, T], F32R, tag=f"rec{i}", name=f"rec{i}")
           for i in range(2)]
    yt = p_y.tile([128, ET, T], BF16)
    qkt = p_qk.tile([128, JQK // 128, T], BF16)
    vt = p_v.tile([128, TT, H, DH + 1], BF16)
    xt = p_x.tile([128, ET, T], BF16)

    nc.sync.dma_start(out=indb, in_=ind)

    def mm_psum(tag):
        return psum.tile([128, 512], F32, tag=tag, bufs=2, name="ps_" + tag)

    # ---- dense generators: qkv projection ---------------------------------
    # Weight loads come in merged [128, 1024] tiles (two 512-wide feature
    # groups per DMA) to halve the descriptor-issue load on the sync queue.
    wqk_tiles = {}                         # (jgpair, et) -> tile

    def load_wqk(jgp, interleave_x=False):
        for et in range(ET):
            if interleave_x:               # interleave x loads with group 0
                nc.sync.dma_start(out=xt[:, et, :],
                                  in_=xT[et * 128:(et + 1) * 128, :])
            wt = p_wqk.tile([128, 1024], BF16, tag="wqk", name="wt")
            nc.sync.dma_start(out=wt, in_=wqkT[et * 128:(et + 1) * 128,
                                              jgp * 1024:(jgp + 1) * 1024])
            wqk_tiles[(jgp, et)] = wt

    def qk_gen(jg, first=False):
        """qkT[j, t] = Wqk x^T + bqk for the 512-wide feature group jg."""
        if first:
            load_wqk(0, interleave_x=True)
            nc.sync.dma_start(out=bqkb, in_=bqk)
            nc.sync.dma_start(out=mask01b, in_=mask01)
            for tt in range(TT):             # fused-denominator ones column
                nc.sync.dma_start(out=vt[:, tt, :, DH], in_=onesbf)
            load_wqk(1)
        jgp, jo = jg // 2, (jg % 2) * 512
        for js in range(4):
            jt = jg * 4 + js
            for th in range(NT):
                ps = mm_psum("mm")
                for et in range(ET):
                    nc.tensor.matmul(
                        ps,
                        wqk_tiles[(jgp, et)][:, jo + js * 128:
                                             jo + (js + 1) * 128],
                        xt[:, et, th * 512:(th + 1) * 512],
                        start=(et == 0), stop=(et == ET - 1))
                    yield
                nc.scalar.activation(
                    out=qkt[:, jt, th * 512:(th + 1) * 512], in_=ps,
                    func=Act.Identity, bias=bqkb[:, jt:jt + 1], scale=1.0)

    def vb_gen(jh):
        """v[t, h, d] token-major for heads 8*jh..8*jh+7 (bias folded into
        c_proj host-side)."""
        wvtiles = []
        if jh == 0:                        # one merged [128, 1024] load set
            for et in range(ET):           # serves both vb passes
                wt = p_wqk.tile([128, 1024], BF16, tag="wv", name="wt",
                                bufs=8)
                nc.sync.dma_start(out=wt, in_=wvT[et * 128:(et + 1) * 128, :])
                wqk_tiles[("wv", et)] = wt
        for tt in range(TT):
            ps = mm_psum("mm")
            for et in range(ET):
                nc.tensor.matmul(
                    ps,
                    xt[:, et, tt * 128:(tt + 1) * 128],
                    wqk_tiles[("wv", et)][:, jh * 512:(jh + 1) * 512],
                    start=(et == 0), stop=(et == ET - 1))
                yield
            nc.vector.tensor_copy(
                out=vt[:, tt, jh * 8:(jh + 1) * 8, 0:DH],
                in_=ps.rearrange("p (h d) -> p h d", d=DH))

    # ---- attention (yields once per tk-iteration) --------------------------
    LAG = 3

    def norm_half(c, half, rb_tags=None):
        """1/den for (chunk c, pair-half) via the fast approx-reciprocal
        (softmax sums are positive, well in range; raw custom-DVE op on the
        fp32 bit layout, which f32r shares), partition-broadcast by K=8
        indicator matmuls; normalizes y in place on DVE."""
        cs = c * 512
        nc.vector._custom_dve(
            RECIPROCAL_APPROX_FAST,
            out=rec[half][0:8, cs:cs + 512], in0=den[half][0:8, cs:cs + 512],
            s0=RC["s0"], s1=RC["s1"], imm2=RC["imm2"])
        for ap in range(4):
            a = half * 4 + ap
            tag, bufs = (rb_tags[ap] if rb_tags else ("mm", 2))
            rb = psum.tile([128, 512], F32, tag=tag, bufs=bufs, name="rb")
            nc.tensor.matmul(
                rb, indb[:, ap * 128:(ap + 1) * 128],
                rec[half][0:8, cs:cs + 512],
                start=True, stop=True)
            nc.vector.tensor_mul(yt[:, a, cs:cs + 512],
                                 yt[:, a, cs:cs + 512], rb)

    def att_gen(c, a, p_esc, p_nrm):
        cs = c * 512
        last_it = 4 * c + 3
        qj = a                             # q tile of the pair
        kj = (JQK // 2) // 128 + a         # k tile of the pair
        avps = [psum.tile([128, 512], F32, tag=f"av{p}", bufs=1,
                          name=f"avp{p}") for p in range(2)]
        pend = []

        def emit_av(it, sub, clen, esc):
            for p in range(2):
                nc.tensor.matmul(
                    avps[p][0:DH + 1, sub:sub + clen],
                    vt[:, it, 2 * a + p, :],
                    esc[:, p, :clen],
                    start=(it == 0), stop=(it == last_it),
                    skip_group_check=True)

        for it in range(last_it + 1):
            n0 = it * 128
            lo = max(n0, cs)
            sub = lo - cs
            clen = 512 - sub
            scp = psum.tile([128, 2, 512], F32, tag="sc", bufs=2, name="scp")
            for p in range(2):             # paired heads: row-tiled matmuls
                pb = p * 64
                nc.tensor.matmul(
                    scp[:, p, :clen],
                    qkt[pb:pb + 64, kj, n0:n0 + 128],
                    qkt[pb:pb + 64, qj, lo:lo + clen],
                    start=True, stop=True)
            esc = p_esc.tile([128, 2, 512], BF16, tag="esc", name="esc")
            nc.scalar.activation(out=esc[:, :, :clen], in_=scp[:, :, :clen],
                                 func=Act.Exp, scale=1.0 / 8.0)
            if n0 >= cs:                   # diagonal block: causal mask on
                nc.gpsimd.tensor_mul(      # the idle gpsimd engine (esc and
                    esc[:, :, 0:128], esc[:, :, 0:128],   # mask are SBUF)
                    mask01b[:, None, :].broadcast_to([128, 2, 128]))
            pend.append((it, sub, clen, esc))
            if len(pend) > LAG:
                emit_av(*pend.pop(0))
            yield
        for args in pend:
            emit_av(*args)
        for p in range(2):                 # drain unnormalized y + denom row
            h = 2 * a + p
            nc.vector.tensor_copy(out=yt[p * 64:p * 64 + 64, qj,
                                         cs:cs + 512],
                                  in_=avps[p][0:DH, :])
            # engines can only address partition bases that are multiples of
            # 32, so stage the denominator row at partition 64 and DMA-
            # scatter it (partition-agnostic, on the gpsimd queue) into the
            # den tile's row.
            stg = p_nrm.tile([128, 512], F32, tag="stg", bufs=2, name="stg")
            nc.vector.tensor_copy(out=stg[64:65, :],
                                  in_=avps[p][DH:DH + 1, :])
            nc.sync.dma_start(out=den[a // 4][h % 8:h % 8 + 1,
                                              cs:cs + 512],
                              in_=stg[64:65, :])

    # ---- drivers ----------------------------------------------------------
    def run_dense(dense, n=None):
        steps = 0
        while dense and (n is None or steps < n):
            try:
                next(dense[0])
                steps += 1
            except StopIteration:
                dense.pop(0)
        return steps

    def drive(att_units, dense, callbacks=None, ratio=5):
        """Round-robin one att unit at a time against the dense stream.
        callbacks[i] (if set) runs right after att unit i completes."""
        att_units = list(att_units)
        callbacks = callbacks or {}
        i = 0
        while att_units:
            try:
                next(att_units[0])
            except StopIteration:
                att_units.pop(0)
                cb = callbacks.pop(i, None)
                if cb:
                    cb()
                i += 1
                continue
            run_dense(dense, ratio)
        run_dense(dense)

    # S1: dense deps for attention pairs 0-3
    run_dense([qk_gen(0, first=True), qk_gen(2), vb_gen(0)])

    # S2: attention pairs 0-3 (both chunks) + pairs 4-7 (chunk 0) over the
    # remaining qkv work; pair-norms trail one unit behind.
    p_esc1 = tc.alloc_tile_pool(name="p_esc1", bufs=4)
    p_nrm1 = tc.alloc_tile_pool(name="p_nrm1", bufs=1)
    nc.sync.dma_start(out=bcb, in_=bc)
    dense2 = [qk_gen(1), qk_gen(3), vb_gen(1)]
    att2 = ([att_gen(c, a, p_esc1, p_nrm1)
             for a in range(4) for c in range(NT)]
            + [att_gen(0, a, p_esc1, p_nrm1) for a in range(4, 8)])
    cbs = {
        8: lambda: norm_half(0, 0),        # after att(c0, pair 4): pairs 0-3
        9: lambda: norm_half(1, 0),        # of both chunks long complete, so
    }                                      # the recips never stall the PE
    drive(att2, dense2, cbs)
    p_nrm1.release()
    p_esc1.release()
    p_wqk.release()
    p_x.release()

    # S3: attention pairs 4-7 (chunk 1) over out_proj + c_proj of chunk 0
    p_w3 = tc.alloc_tile_pool(name="p_w3", bufs=16)
    p_wc = tc.alloc_tile_pool(name="p_wc", bufs=16)
    p_z = tc.alloc_tile_pool(name="p_z", bufs=1)
    p_out = tc.alloc_tile_pool(name="p_out", bufs=2)
    p_esc2 = tc.alloc_tile_pool(name="p_esc2", bufs=4)
    p_nrm2 = tc.alloc_tile_pool(name="p_nrm2", bufs=1)
    zt = p_z.tile([128, ET, T], BF16)
    wout_tiles = []
    for og in range(2):
        for et in range(ET):
            wt = p_w3.tile([128, 512], BF16, tag="w3", name="wt3")
            nc.sync.dma_start(
                out=wt, in_=woutT[et * 128:(et + 1) * 128,
                                  og * 512:(og + 1) * 512])
            wout_tiles.append(wt)
    wc_tiles = []
    for og in range(2):
        for et in range(ET):
            wt = p_wc.tile([128, 512], BF16, tag="wc", name="wtc")
            nc.sync.dma_start(out=wt, in_=wcT[et * 128:(et + 1) * 128,
                                             og * 512:(og + 1) * 512])
            wc_tiles.append(wt)

    def oproj_gen(th):
        for og in range(2):
            for os_ in range(4):
                ot = og * 4 + os_
                ps = mm_psum("mm")
                for et in range(ET):
                    nc.tensor.matmul(
                        ps,
                        wout_tiles[og * ET + et][:, os_ * 128:(os_ + 1) * 128],
                        yt[:, et, th * 512:(th + 1) * 512],
                        start=(et == 0), stop=(et == ET - 1))
                    yield
                nc.vector.tensor_copy(
                    out=zt[:, ot, th * 512:(th + 1) * 512], in_=ps)

    def cproj_gen(og, th):
        for os_ in range(4):
            ot = og * 4 + os_
            ps = mm_psum("mm")
            for et in range(ET):
                nc.tensor.matmul(
                    ps,
                    wc_tiles[og * ET + et][:, os_ * 128:(os_ + 1) * 128],
                    zt[:, et, th * 512:(th + 1) * 512],
                    start=(et == 0), stop=(et == ET - 1))
                yield
            ob = p_out.tile([128, 512], F32, tag="ob", name="ob")
            nc.vector.tensor_scalar_add(out=ob, in0=ps,
                                        scalar1=bcb[:, ot:ot + 1])
            nc.sync.dma_start(
                out=outT[ot * 128:(ot + 1) * 128, th * 512:(th + 1) * 512],
                in_=ob)

    att3 = [att_gen(1, a, p_esc2, p_nrm2) for a in range(4, 8)]
    dense3 = []

    def open_dense3():
        # (c0, heads 8-15) den rows landed at the end of S2; normalizing here
        # keeps the recip chain off the in-order PE queue's critical path,
        # and out_proj/c_proj chunk 0 only emit after their y is final.
        norm_half(0, 1)
        dense3.extend([oproj_gen(0), cproj_gen(0, 0), cproj_gen(1, 0)])

    drive(att3, dense3, {0: open_dense3})

    # S4: final half-norm on 4 distinct PSUM slots (attention is done, so
    # the av slots are free — no write-after-read serialization), then the
    # chunk-1 projections.
    norm_half(1, 1, rb_tags=(("av0", 1), ("av1", 1), ("mm", 2), ("mm", 2)))
    run_dense([oproj_gen(1)])
    run_dense([cproj_gen(0, 1), cproj_gen(1, 1)])
    p_nrm2.release()
    p_esc2.release()
    p_out.release()
    p_z.release()
    p_wc.release()
    p_w3.release()
    p_v.release()
    p_qk.release()
    p_y.release()
    p_den.release()
    psum.release()
    consts.release()


def _build():
    if "nc" in _CACHE:
        return _CACHE["nc"]
    nc = bacc.Bacc("TRN2", target_bir_lowering=False, debug=False,
                   enable_asserts=True, num_devices=8)
    d = nc.dram_tensor
    aps = [
        d("xT", [E, T], BF16, kind="ExternalInput").ap(),
        d("wqkT", [E, JQK], BF16, kind="ExternalInput").ap(),
        d("wvT", [E, E], BF16, kind="ExternalInput").ap(),
        d("bqk", [128, JQK // 128], F32, kind="ExternalInput").ap(),
        d("woutT", [E, E], BF16, kind="ExternalInput").ap(),
        d("wcT", [E, E], BF16, kind="ExternalInput").ap(),
        d("bc", [128, E // 128], F32, kind="ExternalInput").ap(),
        d("mask01", [128, 128], BF16, kind="ExternalInput").ap(),
        d("ind", [8, (H // 4) * 128], F32R, kind="ExternalInput").ap(),
        d("onesbf", [128, H], BF16, kind="ExternalInput").ap(),
        d("outT", [E, T], F32, kind="ExternalOutput").ap(),
    ]
    with tile.TileContext(nc) as tc:
        _emit(nc, tc, aps)
    nc.compile()
    _CACHE["nc"] = nc
    return nc


def _host_inputs(x, in_proj_w, in_proj_b, out_proj_w, out_proj_b,
                 c_proj_w, c_proj_b):
    f = np.float32
    bf = ml_dtypes.bfloat16
    x = np.asarray(x, f)
    in_proj_w = np.asarray(in_proj_w, f)
    in_proj_b = np.asarray(in_proj_b, f)
    out_proj_w = np.asarray(out_proj_w, f)
    out_proj_b = np.asarray(out_proj_b, f)
    c_proj_w = np.asarray(c_proj_w, f)
    c_proj_b = np.asarray(c_proj_b, f)
    # exact algebraic folds: v-bias and out_proj bias ride into c_proj's bias
    #   z = Wout y + (opb + Wout bv);  out = Wc z + cpb
    #   => out = Wc (Wout y) + [cpb + Wc (opb + Wout bv)]
    bout_eff = out_proj_b + out_proj_w @ in_proj_b[JQK:]
    bc_eff = c_proj_b + c_proj_w @ bout_eff
    # indicator for the denominator partition-broadcast:
    # ind[k, ap*128+j] = 1 iff k == 2*ap + j//64   (per half of 8 heads)
    ind = np.zeros((8, (H // 4) * 128), f)
    for ap in range(H // 4):
        ind[2 * ap, ap * 128:ap * 128 + 64] = 1.0
        ind[2 * ap + 1, ap * 128 + 64:(ap + 1) * 128] = 1.0
    shared = {
        "wqkT": np.ascontiguousarray(in_proj_w[:JQK].T).astype(bf),
        "wvT": np.ascontiguousarray(in_proj_w[JQK:].T).astype(bf),
        "bqk": np.ascontiguousarray(in_proj_b[:JQK].reshape(JQK // 128, 128).T),
        "woutT": np.ascontiguousarray(out_proj_w.T).astype(bf),
        "wcT": np.ascontiguousarray(c_proj_w.T).astype(bf),
        "bc": np.ascontiguousarray(bc_eff.reshape(E // 128, 128).T),
        "mask01": np.where(np.arange(128)[None, :] >= np.arange(128)[:, None],
                           f(1.0), f(0.0)).astype(bf),
        "ind": ind,
        "onesbf": np.ones((128, H), bf),
    }
    return [{**shared, "xT": np.ascontiguousarray(x[b].T).astype(bf)}
            for b in range(B)]


def kernel(x, in_proj_w, in_proj_b, out_proj_w, out_proj_b, c_proj_w,
           c_proj_b):
    nc = _build()
    in_maps = _host_inputs(x, in_proj_w, in_proj_b, out_proj_w, out_proj_b,
                           c_proj_w, c_proj_b)
    res = run_bass_kernel_spmd(nc, in_maps, core_ids=list(range(B)),
                               trace=TRACE)
    _CACHE["last_result"] = res
    out = np.stack([res.results[b]["outT"].T for b in range(B)])
    return np.ascontiguousarray(out, dtype=np.float32)


# revision 33
# speedup vs baseline: 1.0011x; 1.0011x over previous
"""Causal self-attention block (qkv proj + 16-head causal attention + out_proj
+ c_proj) on 8 trn2 NeuronCores, data-parallel over the batch (B=8: one batch
element per core).

Layout strategy (per core, batch element b):
  - Activations are kept feature-major [feature, token] on chip so every
    linear layer is a plain   out = W_T.T @ act   matmul chain with the
    (host-pre-transposed) weight as the stationary operand. No on-device
    transposes at all.
  - All projection matmuls run in bf16 (1 PE cycle/row; verified 3.8e-3 rel
    error on the full pipeline vs the 2e-2 budget); PSUM accumulates fp32.
  - Attention computes transposed scores  sT[tk, tq] = k_h.T q_h  per head
    pair (row-tiled K=64 matmuls), exp with no max-subtraction (scores here
    are bounded by a few units), causal mask as a bf16 multiply on the
    diagonal blocks on the otherwise-idle gpsimd engine, and the AV product
    consumes sT directly with token-major V tiles as the stationary operand.
    A fused ones-row in the V operand (M=65) yields the softmax denominator
    for free.
  - Denominator rows are DMA-scattered into per-half [8, T] tiles; 1/den via
    the fast approx-reciprocal DVE op, partition-broadcast by K=8 indicator
    matmuls into PSUM, and normalized into y by DVE multiplies. Each
    half-chunk normalizes as soon as its 4 attention pairs finish, so only
    the (chunk1, heads 8-15) normalization sits on the critical tail.
  - V bias and out_proj bias are folded into the c_proj bias host-side
    (exact algebra), so only the qk bias is applied on chip.
  - Schedule: S1 computes qk pairs 0-3 + V heads 0-7; S2 runs attention for
    pairs 0-3 (both chunks) and pairs 4-7 (chunk 0) over the remaining
    qkv projection work; S3 runs attention pairs 4-7 (chunk 1) over
    out_proj + c_proj of chunk 0; S4 drains out_proj + c_proj of chunk 1.
"""

import sys

if "/opt/trn_rl_repo" not in sys.path:
    sys.path.insert(0, "/opt/trn_rl_repo")

import ml_dtypes
import numpy as np

import concourse.bass as bass  # noqa: F401  (bass types used via tile/bacc)
import concourse.tile as tile
from concourse import bacc, mybir
from concourse.bass_utils import run_bass_kernel_spmd
from concourse.dve_ops import (
    RECIP_APPROX_FAST_CONSTS as RC,
    RECIPROCAL_APPROX_FAST,
)

B, T, E, H = 8, 1024, 1024, 16
DH = E // H          # 64
JQK = 2 * E          # q+k fused feature dim (2048)
F32 = mybir.dt.float32
F32R = mybir.dt.float32r
BF16 = mybir.dt.bfloat16
Act = mybir.ActivationFunctionType

TRACE = False        # test harness flips this for profiled runs
_CACHE = {}


def _emit(nc, tc, aps):
    (xT, wqkT, wvT, bqk, woutT, wcT, bc, mask01, ind, onesbf, outT) = aps
    ET = E // 128     # 8  e-tiles (contraction)
    TT = T // 128     # 8  token tiles
    NT = T // 512     # 2  512-wide token column chunks

    consts = tc.alloc_tile_pool(name="consts", bufs=1)
    mask01b = consts.tile([128, 128], BF16, tag="mask01b")
    bqkb = consts.tile([128, JQK // 128], F32, tag="bqkb")
    bcb = consts.tile([128, E // 128], F32, tag="bcb")
    indb = consts.tile([8, (H // 4) * 128], F32R, tag="indb")

    psum = tc.alloc_tile_pool(name="psum", bufs=1, space="PSUM")
    p_den = tc.alloc_tile_pool(name="p_den", bufs=1)
    p_y = tc.alloc_tile_pool(name="p_y", bufs=1)
    p_qk = tc.alloc_tile_pool(name="p_qk", bufs=1)
    p_v = tc.alloc_tile_pool(name="p_v", bufs=1)
    p_x = tc.alloc_tile_pool(name="p_x", bufs=1)
    p_wqk = tc.alloc_tile_pool(name="p_wqk", bufs=16)
    den = [p_den.tile([8, T], F32, tag=f"den{i}", name=f"den{i}")
           for i in range(2)]
    rec = [p_den.tile([8, T], F32R, tag=f"rec{i}", name=f"rec{i}")
           for i in range(2)]
    yt = p_y.tile([128, ET, T], BF16)
    z0a = p_y.tile([128, ET, 512], BF16, tag="z0a")
    qkt = p_qk.tile([128, JQK // 128, T], BF16)
    vt = p_v.tile([128, TT, H, DH + 1], BF16)
    xt = p_x.tile([128, ET, T], BF16)

    nc.sync.dma_start(out=indb, in_=ind)

    def mm_psum(tag):
        return psum.tile([128, 512], F32, tag=tag, bufs=2, name="ps_" + tag)

    # ---- dense generators: qkv projection ---------------------------------
    def qk_gen(jg, first=False):
        """qkT[j, t] = Wqk x^T + bqk for the 512-wide feature group jg."""
        wtiles = []
        for et in range(ET):
            if first:                      # interleave x loads with group 0
                nc.sync.dma_start(out=xt[:, et, :],
                                  in_=xT[et * 128:(et + 1) * 128, :])
            wt = p_wqk.tile([128, 512], BF16, tag="wqk", name="wt")
            nc.sync.dma_start(out=wt, in_=wqkT[et * 128:(et + 1) * 128,
                                              jg * 512:(jg + 1) * 512])
            wtiles.append(wt)
        if first:
            nc.sync.dma_start(out=bqkb, in_=bqk)
            nc.sync.dma_start(out=mask01b, in_=mask01)
            for tt in range(TT):             # fused-denominator ones column
                nc.sync.dma_start(out=vt[:, tt, :, DH], in_=onesbf)
        for js in range(4):
            jt = jg * 4 + js
            for th in range(NT):
                ps = mm_psum("mm")
                for et in range(ET):
                    nc.tensor.matmul(
                        ps,
                        wtiles[et][:, js * 128:(js + 1) * 128],
                        xt[:, et, th * 512:(th + 1) * 512],
                        start=(et == 0), stop=(et == ET - 1))
                    yield
                nc.scalar.activation(
                    out=qkt[:, jt, th * 512:(th + 1) * 512], in_=ps,
                    func=Act.Identity, bias=bqkb[:, jt:jt + 1], scale=1.0)

    def vb_gen(jh):
        """v[t, h, d] token-major for heads 8*jh..8*jh+7 (bias folded into
        c_proj host-side)."""
        wvtiles = []
        for et in range(ET):
            wt = p_wqk.tile([128, 512], BF16, tag="wqk", name="wt")
            nc.sync.dma_start(out=wt, in_=wvT[et * 128:(et + 1) * 128,
                                             jh * 512:(jh + 1) * 512])
            wvtiles.append(wt)
        for tt in range(TT):
            ps = mm_psum("mm")
            for et in range(ET):
                nc.tensor.matmul(
                    ps,
                    xt[:, et, tt * 128:(tt + 1) * 128],
                    wvtiles[et],
                    start=(et == 0), stop=(et == ET - 1))
                yield
            nc.vector.tensor_copy(
                out=vt[:, tt, jh * 8:(jh + 1) * 8, 0:DH],
                in_=ps.rearrange("p (h d) -> p h d", d=DH))

    # ---- attention (yields once per tk-iteration) --------------------------
    LAG = 3

    def norm_half(c, half, rb_tags=None):
        """1/den for (chunk c, pair-half) via the fast approx-reciprocal
        (softmax sums are positive, well in range; raw custom-DVE op on the
        fp32 bit layout, which f32r shares), partition-broadcast by K=8
        indicator matmuls; normalizes y in place on DVE."""
        cs = c * 512
        nc.vector._custom_dve(
            RECIPROCAL_APPROX_FAST,
            out=rec[half][0:8, cs:cs + 512], in0=den[half][0:8, cs:cs + 512],
            s0=RC["s0"], s1=RC["s1"], imm2=RC["imm2"])
        for ap in range(4):
            a = half * 4 + ap
            tag, bufs = (rb_tags[ap] if rb_tags else ("mm", 2))
            rb = psum.tile([128, 512], F32, tag=tag, bufs=bufs, name="rb")
            nc.tensor.matmul(
                rb, indb[:, ap * 128:(ap + 1) * 128],
                rec[half][0:8, cs:cs + 512],
                start=True, stop=True)
            nc.vector.tensor_mul(yt[:, a, cs:cs + 512],
                                 yt[:, a, cs:cs + 512], rb)

    def att_gen(c, a, p_esc, p_nrm):
        cs = c * 512
        last_it = 4 * c + 3
        qj = a                             # q tile of the pair
        kj = (JQK // 2) // 128 + a         # k tile of the pair
        avps = [psum.tile([128, 512], F32, tag=f"av{p}", bufs=1,
                          name=f"avp{p}") for p in range(2)]
        pend = []

        def emit_av(it, sub, clen, esc):
            for p in range(2):
                nc.tensor.matmul(
                    avps[p][0:DH + 1, sub:sub + clen],
                    vt[:, it, 2 * a + p, :],
                    esc[:, p, :clen],
                    start=(it == 0), stop=(it == last_it),
                    skip_group_check=True)

        for it in range(last_it + 1):
            n0 = it * 128
            lo = max(n0, cs)
            sub = lo - cs
            clen = 512 - sub
            scp = psum.tile([128, 2, 512], F32, tag="sc", bufs=2, name="scp")
            for p in range(2):             # paired heads: row-tiled matmuls
                pb = p * 64
                nc.tensor.matmul(
                    scp[:, p, :clen],
                    qkt[pb:pb + 64, kj, n0:n0 + 128],
                    qkt[pb:pb + 64, qj, lo:lo + clen],
                    start=True, stop=True)
            esc = p_esc.tile([128, 2, 512], BF16, tag="esc", name="esc")
            nc.scalar.activation(out=esc[:, :, :clen], in_=scp[:, :, :clen],
                                 func=Act.Exp, scale=1.0 / 8.0)
            if n0 >= cs:                   # diagonal block: causal mask on
                nc.gpsimd.tensor_mul(      # the idle gpsimd engine (esc and
                    esc[:, :, 0:128], esc[:, :, 0:128],   # mask are SBUF)
                    mask01b[:, None, :].broadcast_to([128, 2, 128]))
            pend.append((it, sub, clen, esc))
            if len(pend) > LAG:
                emit_av(*pend.pop(0))
            yield
        for args in pend:
            emit_av(*args)
        for p in range(2):                 # drain unnormalized y + denom row
            h = 2 * a + p
            nc.vector.tensor_copy(out=yt[p * 64:p * 64 + 64, qj,
                                         cs:cs + 512],
                                  in_=avps[p][0:DH, :])
            # engines can only address partition bases that are multiples of
            # 32, so stage the denominator row at partition 64 and DMA-
            # scatter it (partition-agnostic, on the gpsimd queue) into the
            # den tile's row.
            stg = p_nrm.tile([128, 512], F32, tag="stg", bufs=2, name="stg")
            nc.vector.tensor_copy(out=stg[64:65, :],
                                  in_=avps[p][DH:DH + 1, :])
            nc.sync.dma_start(out=den[a // 4][h % 8:h % 8 + 1,
                                              cs:cs + 512],
                              in_=stg[64:65, :])

    # ---- drivers ----------------------------------------------------------
    def run_dense(dense, n=None):
        steps = 0
        while dense and (n is None or steps < n):
            try:
                next(dense[0])
                steps += 1
            except StopIteration:
                dense.pop(0)
        return steps

    def drive(att_units, dense, callbacks=None, ratio=5):
        """Round-robin one att unit at a time against the dense stream.
        callbacks[i] (if set) runs right after att unit i completes."""
        att_units = list(att_units)
        callbacks = callbacks or {}
        i = 0
        while att_units:
            try:
                next(att_units[0])
            except StopIteration:
                att_units.pop(0)
                cb = callbacks.pop(i, None)
                if cb:
                    cb()
                i += 1
                continue
            run_dense(dense, ratio)
        run_dense(dense)

    # S1: dense deps for attention pairs 0-3
    run_dense([qk_gen(0, first=True), qk_gen(2), vb_gen(0)])

    # S2: attention pairs 0-3 (both chunks) + pairs 4-7 (chunk 0) over the
    # remaining qkv work. Once (chunk0, pairs 0-3) are normalized, the first
    # contraction half of out_proj chunk 0 joins the dense stream to fill
    # the attention-only S2 tail.
    p_w3 = tc.alloc_tile_pool(name="p_w3", bufs=16)
    p_wc = tc.alloc_tile_pool(name="p_wc", bufs=16)
    p_esc1 = tc.alloc_tile_pool(name="p_esc1", bufs=4)
    p_nrm1 = tc.alloc_tile_pool(name="p_nrm1", bufs=1)
    nc.sync.dma_start(out=bcb, in_=bc)
    wout_tiles = []
    for og in range(2):
        for et in range(ET):
            wt = p_w3.tile([128, 512], BF16, tag="w3", name="wt3")
            nc.sync.dma_start(
                out=wt, in_=woutT[et * 128:(et + 1) * 128,
                                  og * 512:(og + 1) * 512])
            wout_tiles.append(wt)
    wc_tiles = []
    for og in range(2):
        for et in range(ET):
            wt = p_wc.tile([128, 512], BF16, tag="wc", name="wtc")
            nc.sync.dma_start(out=wt, in_=wcT[et * 128:(et + 1) * 128,
                                             og * 512:(og + 1) * 512])
            wc_tiles.append(wt)

    def oproj0_half(lo, hi, drain):
        """out_proj chunk 0, contraction tiles [lo, hi); drain(ot, ps)
        finishes each output group."""
        for og in range(2):
            for os_ in range(4):
                ot = og * 4 + os_
                ps = mm_psum("mm")
                for et in range(lo, hi):
                    nc.tensor.matmul(
                        ps,
                        wout_tiles[og * ET + et][:, os_ * 128:(os_ + 1) * 128],
                        yt[:, et, 0:512],
                        start=(et == lo), stop=(et == hi - 1))
                    yield
                drain(ot, ps)

    def oproj0a_gen():
        yield from oproj0_half(
            0, 4, lambda ot, ps: nc.vector.tensor_copy(
                out=z0a[:, ot, :], in_=ps))

    dense2 = [qk_gen(1), qk_gen(3), vb_gen(1)]
    att2 = ([att_gen(c, a, p_esc1, p_nrm1)
             for a in range(4) for c in range(NT)]
            + [att_gen(0, a, p_esc1, p_nrm1) for a in range(4, 8)])
    def cb8():
        norm_half(0, 0)                    # after att(c0, pair 4): pairs 0-3
        dense2.append(oproj0a_gen())       # of both chunks long complete, so
                                           # the recips never stall the PE
    cbs = {8: cb8, 9: lambda: norm_half(1, 0)}
    drive(att2, dense2, cbs)
    p_nrm1.release()
    p_esc1.release()

    # S3: attention pairs 4-7 (chunk 1) over out_proj + c_proj of chunk 0
    p_z = tc.alloc_tile_pool(name="p_z", bufs=1)
    p_out = tc.alloc_tile_pool(name="p_out", bufs=2)
    p_esc2 = tc.alloc_tile_pool(name="p_esc2", bufs=4)
    p_nrm2 = tc.alloc_tile_pool(name="p_nrm2", bufs=1)
    zt = p_z.tile([128, ET, T], BF16)

    def oproj0b_gen():
        # second contraction half; the drain folds in the S2-computed half
        yield from oproj0_half(
            4, ET, lambda ot, ps: nc.vector.tensor_add(
                zt[:, ot, 0:512], ps, z0a[:, ot, :]))

    def oproj_gen(th):
        for og in range(2):
            for os_ in range(4):
                ot = og * 4 + os_
                ps = mm_psum("mm")
                for et in range(ET):
                    nc.tensor.matmul(
                        ps,
                        wout_tiles[og * ET + et][:, os_ * 128:(os_ + 1) * 128],
                        yt[:, et, th * 512:(th + 1) * 512],
                        start=(et == 0), stop=(et == ET - 1))
                    yield
                nc.vector.tensor_copy(
                    out=zt[:, ot, th * 512:(th + 1) * 512], in_=ps)

    def cproj_gen(og, th):
        for os_ in range(4):
            ot = og * 4 + os_
            ps = mm_psum("mm")
            for et in range(ET):
                nc.tensor.matmul(
                    ps,
                    wc_tiles[og * ET + et][:, os_ * 128:(os_ + 1) * 128],
                    zt[:, et, th * 512:(th + 1) * 512],
                    start=(et == 0), stop=(et == ET - 1))
                yield
            ob = p_out.tile([128, 512], F32, tag="ob", name="ob")
            nc.vector.tensor_scalar_add(out=ob, in0=ps,
                                        scalar1=bcb[:, ot:ot + 1])
            nc.sync.dma_start(
                out=outT[ot * 128:(ot + 1) * 128, th * 512:(th + 1) * 512],
                in_=ob)

    att3 = [att_gen(1, a, p_esc2, p_nrm2) for a in range(4, 8)]
    dense3 = []

    def open_dense3():
        # (c0, heads 8-15) den rows landed at the end of S2; normalizing here
        # keeps the recip chain off the in-order PE queue's critical path,
        # and out_proj/c_proj chunk 0 only emit after their y is final.
        norm_half(0, 1)
        dense3.extend([oproj0b_gen(), cproj_gen(0, 0), cproj_gen(1, 0)])

    drive(att3, dense3, {0: open_dense3})

    # S4: final half-norm on 4 distinct PSUM slots (attention is done, so
    # the av slots are free — no write-after-read serialization), then the
    # chunk-1 projections.
    norm_half(1, 1, rb_tags=(("av0", 1), ("av1", 1), ("mm", 2), ("mm", 2)))
    run_dense([oproj_gen(1)])
    run_dense([cproj_gen(0, 1), cproj_gen(1, 1)])
    p_nrm2.release()
    p_esc2.release()
    p_out.release()
    p_z.release()
    p_wc.release()
    p_w3.release()
    p_wqk.release()
    p_x.release()
    p_v.release()
    p_qk.release()
    p_y.release()
    p_den.release()
    psum.release()
    consts.release()


def _build():
    if "nc" in _CACHE:
        return _CACHE["nc"]
    nc = bacc.Bacc("TRN2", target_bir_lowering=False, debug=False,
                   enable_asserts=True, num_devices=8)
    d = nc.dram_tensor
    aps = [
        d("xT", [E, T], BF16, kind="ExternalInput").ap(),
        d("wqkT", [E, JQK], BF16, kind="ExternalInput").ap(),
        d("wvT", [E, E], BF16, kind="ExternalInput").ap(),
        d("bqk", [128, JQK // 128], F32, kind="ExternalInput").ap(),
        d("woutT", [E, E], BF16, kind="ExternalInput").ap(),
        d("wcT", [E, E], BF16, kind="ExternalInput").ap(),
        d("bc", [128, E // 128], F32, kind="ExternalInput").ap(),
        d("mask01", [128, 128], BF16, kind="ExternalInput").ap(),
        d("ind", [8, (H // 4) * 128], F32R, kind="ExternalInput").ap(),
        d("onesbf", [128, H], BF16, kind="ExternalInput").ap(),
        d("outT", [E, T], F32, kind="ExternalOutput").ap(),
    ]
    with tile.TileContext(nc) as tc:
        _emit(nc, tc, aps)
    nc.compile()
    _CACHE["nc"] = nc
    return nc


def _host_inputs(x, in_proj_w, in_proj_b, out_proj_w, out_proj_b,
                 c_proj_w, c_proj_b):
    f = np.float32
    bf = ml_dtypes.bfloat16
    x = np.asarray(x, f)
    in_proj_w = np.asarray(in_proj_w, f)
    in_proj_b = np.asarray(in_proj_b, f)
    out_proj_w = np.asarray(out_proj_w, f)
    out_proj_b = np.asarray(out_proj_b, f)
    c_proj_w = np.asarray(c_proj_w, f)
    c_proj_b = np.asarray(c_proj_b, f)
    # exact algebraic folds: v-bias and out_proj bias ride into c_proj's bias
    #   z = Wout y + (opb + Wout bv);  out = Wc z + cpb
    #   => out = Wc (Wout y) + [cpb + Wc (opb + Wout bv)]
    bout_eff = out_proj_b + out_proj_w @ in_proj_b[JQK:]
    bc_eff = c_proj_b + c_proj_w @ bout_eff
    # indicator for the denominator partition-broadcast:
    # ind[k, ap*128+j] = 1 iff k == 2*ap + j//64   (per half of 8 heads)
    ind = np.zeros((8, (H // 4) * 128), f)
    for ap in range(H // 4):
        ind[2 * ap, ap * 128:ap * 128 + 64] = 1.0
        ind[2 * ap + 1, ap * 128 + 64:(ap + 1) * 128] = 1.0
    shared = {
        "wqkT": np.ascontiguousarray(in_proj_w[:JQK].T).astype(bf),
        "wvT": np.ascontiguousarray(in_proj_w[JQK:].T).astype(bf),
        "bqk": np.ascontiguousarray(in_proj_b[:JQK].reshape(JQK // 128, 128).T),
        "woutT": np.ascontiguousarray(out_proj_w.T).astype(bf),
        "wcT": np.ascontiguousarray(c_proj_w.T).astype(bf),
        "bc": np.ascontiguousarray(bc_eff.reshape(E // 128, 128).T),
        "mask01": np.where(np.arange(128)[None, :] >= np.arange(128)[:, None],
                           f(1.0), f(0.0)).astype(bf),
        "ind": ind,
        "onesbf": np.ones((128, H), bf),
    }
    return [{**shared, "xT": np.ascontiguousarray(x[b].T).astype(bf)}
            for b in range(B)]


def kernel(x, in_proj_w, in_proj_b, out_proj_w, out_proj_b, c_proj_w,
           c_proj_b):
    nc = _build()
    in_maps = _host_inputs(x, in_proj_w, in_proj_b, out_proj_w, out_proj_b,
                           c_proj_w, c_proj_b)
    res = run_bass_kernel_spmd(nc, in_maps, core_ids=list(range(B)),
                               trace=TRACE)
    _CACHE["last_result"] = res
    out = np.stack([res.results[b]["outT"].T for b in range(B)])
    return np.ascontiguousarray(out, dtype=np.float32)
